# revision 1
# baseline (speedup 1.0000x reference)
"""Trainium2 Bass kernel for nn_Attention_75453985457143 (EfficientViT-style
attention block: 1x1 conv QKV + BN, depthwise 3x3 on Q + BN, MHSA with relative
position bias, ReLU, 1x1 proj + BN).

Data-parallel over batch: 128 images -> 16 per NeuronCore across 8 cores.
All BN affine transforms are folded into weights/bias vectors on the host.
"""

import os
import numpy as np

# ---- problem constants (hardcoded; kernel.py must be self-contained) ----
B = 128
C = 384
KD = 32
NH = 12
NHKD = 384          # q/k channels
DH = 1536           # v channels
RES = 14
N = RES * RES       # 196 tokens
EPS = 1e-5
NCORES = 8
BPC = B // NCORES   # 16 images per core
G = 2               # images per group (pair)
NG = BPC // G       # 8 groups
MT = 98             # attention m-tile (2 tiles of 98 = 196)

_cache = {}


def _build_nc(ng=NG, dbg=False, stage=99):
    import concourse.bacc as bacc
    import concourse.tile as tile
    from concourse import mybir
    from concourse.alu_op_type import AluOpType
    from contextlib import ExitStack

    f32 = mybir.dt.float32
    bf16 = mybir.dt.bfloat16
    AF = mybir.ActivationFunctionType

    nc = bacc.Bacc("TRN2", target_bir_lowering=False, debug=False, num_devices=NCORES)

    # ---- DRAM I/O ----
    x_d = nc.dram_tensor("x", [BPC, C, N], f32, kind="ExternalInput")
    wqk_d = nc.dram_tensor("wqkT", [C, 2 * NHKD], f32, kind="ExternalInput")
    wv_d = nc.dram_tensor("wvT", [C, DH], f32, kind="ExternalInput")
    wp_d = nc.dram_tensor("wpT", [DH, C], f32, kind="ExternalInput")
    biasT_d = nc.dram_tensor("biasT", [2, MT, NH * N], f32, kind="ExternalInput")
    tq_d = nc.dram_tensor("tq", [128, 3], f32, kind="ExternalInput")
    tdw_d = nc.dram_tensor("tdw", [128, 3], f32, kind="ExternalInput")
    wtap_d = nc.dram_tensor("wtap", [128, 27], f32, kind="ExternalInput")
    tv_d = nc.dram_tensor("tv", [128, NH], f32, kind="ExternalInput")
    tp_d = nc.dram_tensor("tp", [128, 3], f32, kind="ExternalInput")
    out_d = nc.dram_tensor("out", [BPC, C, N], f32, kind="ExternalOutput")
    if dbg:
        dbg_qpad = nc.dram_tensor("dbg_qpad", [3, 128, G, 256], f32, kind="ExternalOutput")
        dbg_qc = nc.dram_tensor("dbg_qc", [3, 128, G, N], f32, kind="ExternalOutput")
        dbg_k = nc.dram_tensor("dbg_k", [3, 128, G, N], f32, kind="ExternalOutput")
        dbg_vT = nc.dram_tensor("dbg_vT", [2, MT, DH], f32, kind="ExternalOutput")
        dbg_E = nc.dram_tensor("dbg_E", [2, MT, NH * N], f32, kind="ExternalOutput")
        dbg_Z = nc.dram_tensor("dbg_Z", [NH, N], f32, kind="ExternalOutput")
        dbg_relu = nc.dram_tensor("dbg_relu", [NH, 128, N], f32, kind="ExternalOutput")

    with tile.TileContext(nc) as tc, ExitStack() as ctx:
        singles = ctx.enter_context(tc.tile_pool(name="singles", bufs=1))
        grp2 = ctx.enter_context(tc.tile_pool(name="grp2", bufs=2))
        grp1 = ctx.enter_context(tc.tile_pool(name="grp1", bufs=1))
        imgp = ctx.enter_context(tc.tile_pool(name="imgp", bufs=2))
        accp = ctx.enter_context(tc.tile_pool(name="accp", bufs=1))
        zp = ctx.enter_context(tc.tile_pool(name="zp", bufs=1))
        small = ctx.enter_context(tc.tile_pool(name="small", bufs=3))
        regp = ctx.enter_context(tc.tile_pool(name="regp", bufs=1))
        relup = ctx.enter_context(tc.tile_pool(name="relup", bufs=1))
        ps = ctx.enter_context(tc.tile_pool(name="ps", bufs=2, space="PSUM"))
        ps2 = ctx.enter_context(tc.tile_pool(name="ps2", bufs=6, space="PSUM"))
        dramp = ctx.enter_context(tc.tile_pool(name="dramp", bufs=2, space="DRAM"))

        # ---- persistent constants ----
        wqk_sb = []
        wv_sb = []
        for kt in range(3):
            t = singles.tile([128, 2 * NHKD], f32, tag=f"wqk{kt}")
            nc.sync.dma_start(out=t[:, :], in_=wqk_d[kt * 128:(kt + 1) * 128, :])
            wqk_sb.append(t)
            t = singles.tile([128, DH], f32, tag=f"wv{kt}")
            nc.sync.dma_start(out=t[:, :], in_=wv_d[kt * 128:(kt + 1) * 128, :])
            wv_sb.append(t)
        wp_sb = []
        for kt in range(NH):
            t = singles.tile([128, C], f32, tag=f"wp{kt}")
            nc.sync.dma_start(out=t[:, :], in_=wp_d[kt * 128:(kt + 1) * 128, :])
            wp_sb.append(t)
        biasT_sb = []
        for mt2 in range(2):
            t = singles.tile([MT, NH * N], f32, tag=f"biasT{mt2}")
            nc.sync.dma_start(out=t[:, :], in_=biasT_d[mt2])
            biasT_sb.append(t)
        tq_sb = singles.tile([128, 3], f32, tag="tq")
        nc.sync.dma_start(out=tq_sb[:, :], in_=tq_d[:, :])
        tdw_sb = singles.tile([128, 3], f32, tag="tdw")
        nc.sync.dma_start(out=tdw_sb[:, :], in_=tdw_d[:, :])
        wtap_sb = singles.tile([128, 27], f32, tag="wtap")
        nc.sync.dma_start(out=wtap_sb[:, :], in_=wtap_d[:, :])
        tv_sb = singles.tile([128, NH], f32, tag="tv")
        nc.sync.dma_start(out=tv_sb[:, :], in_=tv_d[:, :])
        tp_sb = singles.tile([128, 3], f32, tag="tp")
        nc.sync.dma_start(out=tp_sb[:, :], in_=tp_d[:, :])
        ones98 = singles.tile([MT, 1], bf16, tag="ones98")
        nc.vector.memset(ones98[:, :], 1.0)

        for g in range(ng):
            i0 = g * G
            # ---------- phase A: load x, qkv matmuls ----------
            x_sb = []
            for kt in range(3):
                t = grp2.tile([128, G, N], f32, tag=f"x{kt}")
                nc.sync.dma_start(
                    out=t[:, :, :],
                    in_=x_d[i0:i0 + G, kt * 128:(kt + 1) * 128, :].rearrange(
                        "g c n -> c g n"),
                )
                x_sb.append(t)
            k_sb = []
            qpad = []
            for pt in range(3):
                t = grp2.tile([128, G, N], bf16, tag=f"k{pt}")
                k_sb.append(t)
                t = grp1.tile([128, G, 16, 16], f32, tag=f"qpad{pt}")
                nc.vector.memset(t[:, :, :, :], 0.0)
                qpad.append(t)

            for mt in range(6):
                qk_ps = ps.tile([128, G * N], f32, tag="ps")
                for kt in range(3):
                    nc.tensor.matmul(
                        qk_ps[:, :],
                        wqk_sb[kt][:, mt * 128:(mt + 1) * 128],
                        x_sb[kt][:, :, :],
                        start=(kt == 0),
                        stop=(kt == 2),
                    )
                if mt < 3:
                    # q: add BN bias, write into padded interior
                    for i in range(G):
                        nc.scalar.activation(
                            qpad[mt][:, i, 1:15, 1:15],
                            qk_ps[:, i * N:(i + 1) * N].rearrange(
                                "p (a b) -> p a b", a=RES),
                            AF.Identity,
                            bias=tq_sb[:, mt:mt + 1],
                        )
                else:
                    nc.any.tensor_copy(
                        k_sb[mt - 3][:, :, :],
                        qk_ps[:, :].rearrange("p (g n) -> p g n", g=G),
                    )

            # ---------- phase B: depthwise 3x3 conv on q ----------
            qconv = []
            for pt in range(3):
                qc = grp1.tile([128, G, RES, RES], bf16, tag=f"qconv{pt}")
                for i in range(G):
                    acc_prev = None
                    for j in range(9):
                        jr, jc = j // 3, j % 3
                        win = qpad[pt][:, i, jr:jr + RES, jc:jc + RES]
                        w_ap = wtap_sb[:, pt * 9 + j:pt * 9 + j + 1]
                        if j == 8:
                            dst = qc[:, i]
                        else:
                            acc_t = accp.tile([128, RES, RES], f32,
                                              tag=f"acc{pt}_{j % 2}")
                            dst = acc_t[:, :, :]
                        if j == 0:
                            nc.vector.tensor_scalar(
                                dst, win, w_ap,
                                tdw_sb[:, pt:pt + 1],
                                AluOpType.mult, AluOpType.add)
                        else:
                            nc.vector.scalar_tensor_tensor(
                                dst, win, w_ap, acc_prev,
                                AluOpType.mult, AluOpType.add)
                        acc_prev = dst
                qconv.append(qc)

            if dbg and g == 0:
                for pt in range(3):
                    nc.sync.dma_start(out=dbg_qpad.ap()[pt], in_=qpad[pt][:, :, :, :].rearrange("p g a b -> p g (a b)"))
                    nc.sync.dma_start(out=dbg_qc.ap()[pt], in_=qconv[pt][:, :, :, :].rearrange("p g a b -> p g (a b)"))
                    nc.sync.dma_start(out=dbg_k.ap()[pt], in_=k_sb[pt][:, :, :])

            # ---------- regroup k/qconv to base-partition-0 head layout ----------
            k2 = regp.tile([32, NH, G, N], bf16, tag="k2")
            q2 = regp.tile([32, NH, G, N], bf16, tag="q2")
            for pt in range(3):
                for r in range(4):
                    h = 4 * pt + r
                    nc.sync.dma_start(
                        out=k2[:, h, :, :],
                        in_=k_sb[pt][32 * r:32 * r + 32, :, :])
                    nc.sync.dma_start(
                        out=q2[:, h, :, :],
                        in_=qconv[pt][32 * r:32 * r + 32, :, :, :].rearrange(
                            "d g a b -> d g (a b)"))

            # ---------- phase C: per-image attention ----------
            relu_t = [[None] * NH for _ in range(G)]
            for i in range(G):
                if stage < 2:
                    continue
                # v^T: [196, 1536] via x-stationary matmuls
                vT_sb = []
                for mt2 in range(2):
                    vt = imgp.tile([MT, DH], bf16, tag=f"vT{mt2}")
                    for ch in range(3):
                        vps = ps.tile([MT, 512], f32, tag="ps")
                        for kt in range(3):
                            nc.tensor.matmul(
                                vps[:, :],
                                x_sb[kt][:, i, mt2 * MT:(mt2 + 1) * MT],
                                wv_sb[kt][:, ch * 512:(ch + 1) * 512],
                                start=(kt == 0),
                                stop=(kt == 2),
                            )
                        nc.any.tensor_copy(vt[:, ch * 512:(ch + 1) * 512], vps[:, :])
                    vT_sb.append(vt)
                if dbg and g == 0 and i == 0:
                    for mt2 in range(2):
                        nc.sync.dma_start(out=dbg_vT.ap()[mt2], in_=vT_sb[mt2][:, :])

                if stage < 3:
                    continue
                # QK + bias + exp (E^T layout [m, n], head pairs packed in free)
                E_sb = []
                for mt2 in range(2):
                    et = imgp.tile([MT, NH * N], bf16, tag=f"E{mt2}")
                    E_sb.append(et)
                for mt2 in range(2):
                    for hp in range(6):
                        sps = ps2.tile([MT, 2 * N], f32, tag="ps2")
                        for hh in range(2):
                            h = 2 * hp + hh
                            nc.tensor.matmul(
                                sps[:, hh * N:(hh + 1) * N],
                                k2[:, h, i, mt2 * MT:(mt2 + 1) * MT],
                                q2[:, h, i, :],
                                start=True,
                                stop=True,
                            )
                        tmp = small.tile([MT, 2 * N], f32, tag="stmp")
                        nc.vector.tensor_add(
                            tmp[:, :], sps[:, :],
                            biasT_sb[mt2][:, hp * 2 * N:(hp + 1) * 2 * N])
                        nc.scalar.activation(
                            E_sb[mt2][:, hp * 2 * N:(hp + 1) * 2 * N],
                            tmp[:, :], AF.Exp)

                if stage < 4:
                    continue
                # Z = colsums of E (per head) via ones-stationary matmuls
                Z1 = zp.tile([1, NH, N], f32, tag="Z1")
                for hp in range(6):
                    zps = ps2.tile([1, 2 * N], f32, tag="ps2")
                    for hh in range(2):
                        h = 2 * hp + hh
                        for mt2 in range(2):
                            nc.tensor.matmul(
                                zps[:, hh * N:(hh + 1) * N],
                                ones98[:, :],
                                E_sb[mt2][:, h * N:(h + 1) * N],
                                start=(mt2 == 0),
                                stop=(mt2 == 1),
                            )
                    nc.any.tensor_copy(
                        Z1[:, 2 * hp:2 * hp + 2, :],
                        zps[:, :].rearrange("p (a n) -> p a n", a=2))
                # shuffle [1, 12*196] -> [12, 196] so reciprocal gets 12 lanes
                Z12 = zp.tile([NH, N], f32, tag="Z12")
                nc.sync.dma_start(out=Z12[:, :], in_=Z1[:, :, :])
                invZ = zp.tile([NH, N], f32, tag="invZ")
                nc.vector.reciprocal(invZ[:, :], Z12[:, :])
                if dbg and g == 0 and i == 0:
                    for mt2 in range(2):
                        nc.sync.dma_start(out=dbg_E.ap()[mt2], in_=E_sb[mt2][:, :])
                    nc.sync.dma_start(out=dbg_Z.ap()[:, :], in_=Z12[:, :])
                invZd = dramp.tile([NH, N], f32, tag="invZd")
                nc.sync.dma_start(out=invZd[:, :], in_=invZ[:, :])

                if stage < 5:
                    continue
                # AV + normalize + relu
                for h in range(NH):
                    rps = ps2.tile([128, N], f32, tag="ps2")
                    for mt2 in range(2):
                        nc.tensor.matmul(
                            rps[:, :],
                            vT_sb[mt2][:, h * 128:(h + 1) * 128],
                            E_sb[mt2][:, h * N:(h + 1) * N],
                            start=(mt2 == 0),
                            stop=(mt2 == 1),
                        )
                    invZb = small.tile([128, N], f32, tag="invZb")
                    nc.sync.dma_start(
                        out=invZb[:, :],
                        in_=invZd[h:h + 1, :].to_broadcast([128, N]))
                    tmp2 = small.tile([128, N], f32, tag="avtmp")
                    nc.vector.tensor_mul(tmp2[:, :], rps[:, :], invZb[:, :])
                    if i == 0:
                        rt = relup.tile([128, G, N], f32, tag=f"relu{h}")
                        relu_t[0][h] = rt
                    else:
                        rt = relu_t[0][h]
                    nc.scalar.activation(
                        rt[:, i, :], tmp2[:, :], AF.Relu, bias=tv_sb[:, h:h + 1])
                    if dbg and g == 0 and i == 0:
                        nc.sync.dma_start(out=dbg_relu.ap()[h], in_=rt[:, :])

            # ---------- proj (pair-batched) + BN bias + store ----------
            if stage < 6:
                continue
            for mt in range(3):
                mps = ps.tile([128, G * N], f32, tag="ps")
                for kt in range(NH):
                    nc.tensor.matmul(
                        mps[:, :],
                        wp_sb[kt][:, mt * 128:(mt + 1) * 128],
                        relu_t[0][kt][:, :, :],
                        start=(kt == 0),
                        stop=(kt == NH - 1),
                    )
                o_sb = small.tile([128, G * N], f32, tag="osb")
                nc.vector.tensor_scalar_add(o_sb[:, :], mps[:, :], tp_sb[:, mt:mt + 1])
                for i in range(G):
                    nc.sync.dma_start(
                        out=out_d[i0 + i, mt * 128:(mt + 1) * 128, :],
                        in_=o_sb[:, i * N:(i + 1) * N],
                    )

    nc.finalize()
    return nc


def _host_prep(inputs):
    inp = {k: np.asarray(v, dtype=np.float32) if np.asarray(v).dtype != np.int32
           else np.asarray(v) for k, v in inputs.items()}
    x = inp["x"].reshape(B, C, N)

    s_qkv = inp["qkv_g"] / np.sqrt(inp["qkv_v"] + EPS)
    t_qkv = inp["qkv_b"] - inp["qkv_m"] * s_qkv
    W = inp["qkv_w"][:, :, 0, 0] * s_qkv[:, None]          # [2304, 384]
    Wq = W[:NHKD]
    Wk = W[NHKD:2 * NHKD] * (KD ** -0.5)
    Wv = W[2 * NHKD:]
    tq = t_qkv[:NHKD]
    tv = t_qkv[2 * NHKD:]
    wqkT = np.ascontiguousarray(np.concatenate([Wq, Wk], 0).T)   # [384, 768]
    wvT = np.ascontiguousarray(Wv.T)                             # [384, 1536]

    s_dw = inp["dw_g"] / np.sqrt(inp["dw_v"] + EPS)
    tdw = inp["dw_b"] - inp["dw_m"] * s_dw
    wtap = inp["dw_w"][:, 0].reshape(NHKD, 9) * s_dw[:, None]    # [384, 9]

    s_p = inp["proj_g"] / np.sqrt(inp["proj_v"] + EPS)
    tp = inp["proj_b"] - inp["proj_m"] * s_p
    wpT = np.ascontiguousarray((inp["proj_w"][:, :, 0, 0] * s_p[:, None]).T)

    bias_full = np.take(inp["attn_biases"], inp["bias_idxs"], axis=1)  # [12,n,m]
    bias_m = bias_full.transpose(0, 2, 1)                               # [12,m,n]
    biasT = np.ascontiguousarray(
        bias_m.reshape(NH, 2, MT, N).transpose(1, 2, 0, 3).reshape(2, MT, NH * N))

    def col(v):   # [384] -> [128, 3]
        return np.ascontiguousarray(v.reshape(3, 128).T)

    feed = {
        "wqkT": wqkT.astype(np.float32),
        "wvT": wvT.astype(np.float32),
        "wpT": wpT.astype(np.float32),
        "biasT": biasT.astype(np.float32),
        "tq": col(tq).astype(np.float32),
        "tdw": col(tdw).astype(np.float32),
        "wtap": np.ascontiguousarray(
            wtap.reshape(3, 128, 9).transpose(1, 0, 2).reshape(128, 27)
        ).astype(np.float32),
        "tv": np.ascontiguousarray(tv.reshape(NH, 128).T).astype(np.float32),
        "tp": col(tp).astype(np.float32),
    }
    return x, feed


def get_nc():
    if "nc" not in _cache:
        _cache["nc"] = _build_nc()
    return _cache["nc"]


def kernel(**inputs) -> np.ndarray:
    from concourse.bass_utils import run_bass_kernel_spmd

    x, feed = _host_prep(inputs)
    nc = get_nc()
    in_maps = []
    for c in range(NCORES):
        m = dict(feed)
        m["x"] = np.ascontiguousarray(x[c * BPC:(c + 1) * BPC])
        in_maps.append(m)
    res = run_bass_kernel_spmd(nc, in_maps, core_ids=list(range(NCORES)))
    out = np.concatenate([res.results[c]["out"] for c in range(NCORES)], axis=0)
    return out.reshape(B, C, RES, RES)



# revision 2
# speedup vs baseline: 4.0726x; 4.0726x over previous
"""Trainium2 Bass kernel for nn_Attention_75453985457143 (EfficientViT-style
attention block: 1x1 conv QKV + BN, depthwise 3x3 on Q + BN, MHSA with relative
position bias, ReLU, 1x1 proj + BN).

Data-parallel over batch: 128 images -> 16 per NeuronCore across 8 cores.
All BN affine transforms are folded into weights/bias vectors on the host.

The wall-clock cost of a call is dominated by host<->device transfer over the
axon relay (~55-65 MB/s, serialized), so the runtime path is organized around
minimizing transferred bytes per call:
  - x is uploaded in bf16 (19.25 MB instead of 38.5 MB f32)
  - the output comes back in bf16 and is cast to f32 on the host
  - all weights are uploaded to the devices once and kept resident
  - the donated output buffer is recycled from the previous call's output
    instead of uploading fresh zero buffers every call (the kernel writes
    every element of out, so the initial contents are irrelevant)
"""

import time
import numpy as np

# ---- problem constants (hardcoded; kernel.py must be self-contained) ----
B = 128
C = 384
KD = 32
NH = 12
NHKD = 384          # q/k channels
DH = 1536           # v channels
RES = 14
N = RES * RES       # 196 tokens
EPS = 1e-5
NCORES = 8
BPC = B // NCORES   # 16 images per core
G = 2               # images per group (pair)
NG = BPC // G       # 8 groups
MT = 98             # attention m-tile (2 tiles of 98 = 196)

_cache = {}


def _build_nc(ng=NG):
    import concourse.bacc as bacc
    import concourse.tile as tile
    from concourse import mybir
    from concourse.alu_op_type import AluOpType
    from contextlib import ExitStack

    f32 = mybir.dt.float32
    bf16 = mybir.dt.bfloat16
    AF = mybir.ActivationFunctionType

    nc = bacc.Bacc("TRN2", target_bir_lowering=False, debug=False, num_devices=NCORES)

    # ---- DRAM I/O ----
    x_d = nc.dram_tensor("x", [BPC, C, N], bf16, kind="ExternalInput")
    wqk_d = nc.dram_tensor("wqkT", [C, 2 * NHKD], bf16, kind="ExternalInput")
    wv_d = nc.dram_tensor("wvT", [C, DH], bf16, kind="ExternalInput")
    wp_d = nc.dram_tensor("wpT", [DH, C], f32, kind="ExternalInput")
    biasT_d = nc.dram_tensor("biasT", [2, MT, NH * N], f32, kind="ExternalInput")
    tq_d = nc.dram_tensor("tq", [128, 3], f32, kind="ExternalInput")
    tdw_d = nc.dram_tensor("tdw", [128, 3], f32, kind="ExternalInput")
    wtap_d = nc.dram_tensor("wtap", [128, 27], f32, kind="ExternalInput")
    tv_d = nc.dram_tensor("tv", [128, NH], f32, kind="ExternalInput")
    tp_d = nc.dram_tensor("tp", [128, 3], f32, kind="ExternalInput")
    out_d = nc.dram_tensor("out", [BPC, C, N], bf16, kind="ExternalOutput")

    with tile.TileContext(nc) as tc, ExitStack() as ctx:
        singles = ctx.enter_context(tc.tile_pool(name="singles", bufs=1))
        grp2 = ctx.enter_context(tc.tile_pool(name="grp2", bufs=2))
        grp1 = ctx.enter_context(tc.tile_pool(name="grp1", bufs=1))
        imgp = ctx.enter_context(tc.tile_pool(name="imgp", bufs=2))
        accp = ctx.enter_context(tc.tile_pool(name="accp", bufs=1))
        zp = ctx.enter_context(tc.tile_pool(name="zp", bufs=1))
        small = ctx.enter_context(tc.tile_pool(name="small", bufs=3))
        regp = ctx.enter_context(tc.tile_pool(name="regp", bufs=1))
        relup = ctx.enter_context(tc.tile_pool(name="relup", bufs=1))
        ps = ctx.enter_context(tc.tile_pool(name="ps", bufs=2, space="PSUM"))
        ps2 = ctx.enter_context(tc.tile_pool(name="ps2", bufs=6, space="PSUM"))
        dramp = ctx.enter_context(tc.tile_pool(name="dramp", bufs=2, space="DRAM"))

        # ---- persistent constants ----
        wqk_sb = []
        wv_sb = []
        for kt in range(3):
            t = singles.tile([128, 2 * NHKD], bf16, tag=f"wqk{kt}")
            nc.sync.dma_start(out=t[:, :], in_=wqk_d[kt * 128:(kt + 1) * 128, :])
            wqk_sb.append(t)
            t = singles.tile([128, DH], bf16, tag=f"wv{kt}")
            nc.sync.dma_start(out=t[:, :], in_=wv_d[kt * 128:(kt + 1) * 128, :])
            wv_sb.append(t)
        wp_sb = []
        for kt in range(NH):
            t = singles.tile([128, C], f32, tag=f"wp{kt}")
            nc.sync.dma_start(out=t[:, :], in_=wp_d[kt * 128:(kt + 1) * 128, :])
            wp_sb.append(t)
        biasT_sb = []
        for mt2 in range(2):
            t = singles.tile([MT, NH * N], f32, tag=f"biasT{mt2}")
            nc.sync.dma_start(out=t[:, :], in_=biasT_d[mt2])
            biasT_sb.append(t)
        tq_sb = singles.tile([128, 3], f32, tag="tq")
        nc.sync.dma_start(out=tq_sb[:, :], in_=tq_d[:, :])
        tdw_sb = singles.tile([128, 3], f32, tag="tdw")
        nc.sync.dma_start(out=tdw_sb[:, :], in_=tdw_d[:, :])
        wtap_sb = singles.tile([128, 27], f32, tag="wtap")
        nc.sync.dma_start(out=wtap_sb[:, :], in_=wtap_d[:, :])
        tv_sb = singles.tile([128, NH], f32, tag="tv")
        nc.sync.dma_start(out=tv_sb[:, :], in_=tv_d[:, :])
        tp_sb = singles.tile([128, 3], f32, tag="tp")
        nc.sync.dma_start(out=tp_sb[:, :], in_=tp_d[:, :])
        ones98 = singles.tile([MT, 1], bf16, tag="ones98")
        nc.vector.memset(ones98[:, :], 1.0)

        for g in range(ng):
            i0 = g * G
            # ---------- phase A: load x, qkv matmuls ----------
            x_sb = []
            for kt in range(3):
                t = grp2.tile([128, G, N], bf16, tag=f"x{kt}")
                nc.sync.dma_start(
                    out=t[:, :, :],
                    in_=x_d[i0:i0 + G, kt * 128:(kt + 1) * 128, :].rearrange(
                        "g c n -> c g n"),
                )
                x_sb.append(t)
            k_sb = []
            qpad = []
            for pt in range(3):
                t = grp2.tile([128, G, N], bf16, tag=f"k{pt}")
                k_sb.append(t)
                t = grp1.tile([128, G, 16, 16], f32, tag=f"qpad{pt}")
                nc.vector.memset(t[:, :, :, :], 0.0)
                qpad.append(t)

            for mt in range(6):
                qk_ps = ps.tile([128, G * N], f32, tag="ps")
                for kt in range(3):
                    nc.tensor.matmul(
                        qk_ps[:, :],
                        wqk_sb[kt][:, mt * 128:(mt + 1) * 128],
                        x_sb[kt][:, :, :],
                        start=(kt == 0),
                        stop=(kt == 2),
                    )
                if mt < 3:
                    # q: add BN bias, write into padded interior
                    for i in range(G):
                        nc.scalar.activation(
                            qpad[mt][:, i, 1:15, 1:15],
                            qk_ps[:, i * N:(i + 1) * N].rearrange(
                                "p (a b) -> p a b", a=RES),
                            AF.Identity,
                            bias=tq_sb[:, mt:mt + 1],
                        )
                else:
                    nc.any.tensor_copy(
                        k_sb[mt - 3][:, :, :],
                        qk_ps[:, :].rearrange("p (g n) -> p g n", g=G),
                    )

            # ---------- phase B: depthwise 3x3 conv on q ----------
            qconv = []
            for pt in range(3):
                qc = grp1.tile([128, G, RES, RES], bf16, tag=f"qconv{pt}")
                for i in range(G):
                    acc_prev = None
                    for j in range(9):
                        jr, jc = j // 3, j % 3
                        win = qpad[pt][:, i, jr:jr + RES, jc:jc + RES]
                        w_ap = wtap_sb[:, pt * 9 + j:pt * 9 + j + 1]
                        if j == 8:
                            dst = qc[:, i]
                        else:
                            acc_t = accp.tile([128, RES, RES], f32,
                                              tag=f"acc{pt}_{j % 2}")
                            dst = acc_t[:, :, :]
                        if j == 0:
                            nc.vector.tensor_scalar(
                                dst, win, w_ap,
                                tdw_sb[:, pt:pt + 1],
                                AluOpType.mult, AluOpType.add)
                        else:
                            nc.vector.scalar_tensor_tensor(
                                dst, win, w_ap, acc_prev,
                                AluOpType.mult, AluOpType.add)
                        acc_prev = dst
                qconv.append(qc)

            # ---------- regroup k/qconv to base-partition-0 head layout ----------
            k2 = regp.tile([32, NH, G, N], bf16, tag="k2")
            q2 = regp.tile([32, NH, G, N], bf16, tag="q2")
            for pt in range(3):
                for r in range(4):
                    h = 4 * pt + r
                    nc.sync.dma_start(
                        out=k2[:, h, :, :],
                        in_=k_sb[pt][32 * r:32 * r + 32, :, :])
                    nc.sync.dma_start(
                        out=q2[:, h, :, :],
                        in_=qconv[pt][32 * r:32 * r + 32, :, :, :].rearrange(
                            "d g a b -> d g (a b)"))

            # ---------- phase C: per-image attention ----------
            relu_t = [[None] * NH for _ in range(G)]
            for i in range(G):
                # v^T: [196, 1536] via x-stationary matmuls
                vT_sb = []
                for mt2 in range(2):
                    vt = imgp.tile([MT, DH], bf16, tag=f"vT{mt2}")
                    for ch in range(3):
                        vps = ps.tile([MT, 512], f32, tag="ps")
                        for kt in range(3):
                            nc.tensor.matmul(
                                vps[:, :],
                                x_sb[kt][:, i, mt2 * MT:(mt2 + 1) * MT],
                                wv_sb[kt][:, ch * 512:(ch + 1) * 512],
                                start=(kt == 0),
                                stop=(kt == 2),
                            )
                        nc.any.tensor_copy(vt[:, ch * 512:(ch + 1) * 512], vps[:, :])
                    vT_sb.append(vt)

                # QK + bias + exp (E^T layout [m, n], head pairs packed in free)
                E_sb = []
                for mt2 in range(2):
                    et = imgp.tile([MT, NH * N], bf16, tag=f"E{mt2}")
                    E_sb.append(et)
                for mt2 in range(2):
                    for hp in range(6):
                        sps = ps2.tile([MT, 2 * N], f32, tag="ps2")
                        for hh in range(2):
                            h = 2 * hp + hh
                            nc.tensor.matmul(
                                sps[:, hh * N:(hh + 1) * N],
                                k2[:, h, i, mt2 * MT:(mt2 + 1) * MT],
                                q2[:, h, i, :],
                                start=True,
                                stop=True,
                            )
                        tmp = small.tile([MT, 2 * N], f32, tag="stmp")
                        nc.vector.tensor_add(
                            tmp[:, :], sps[:, :],
                            biasT_sb[mt2][:, hp * 2 * N:(hp + 1) * 2 * N])
                        nc.scalar.activation(
                            E_sb[mt2][:, hp * 2 * N:(hp + 1) * 2 * N],
                            tmp[:, :], AF.Exp)

                # Z = colsums of E (per head) via ones-stationary matmuls
                Z1 = zp.tile([1, NH, N], f32, tag="Z1")
                for hp in range(6):
                    zps = ps2.tile([1, 2 * N], f32, tag="ps2")
                    for hh in range(2):
                        h = 2 * hp + hh
                        for mt2 in range(2):
                            nc.tensor.matmul(
                                zps[:, hh * N:(hh + 1) * N],
                                ones98[:, :],
                                E_sb[mt2][:, h * N:(h + 1) * N],
                                start=(mt2 == 0),
                                stop=(mt2 == 1),
                            )
                    nc.any.tensor_copy(
                        Z1[:, 2 * hp:2 * hp + 2, :],
                        zps[:, :].rearrange("p (a n) -> p a n", a=2))
                # shuffle [1, 12*196] -> [12, 196] so reciprocal gets 12 lanes
                Z12 = zp.tile([NH, N], f32, tag="Z12")
                nc.sync.dma_start(out=Z12[:, :], in_=Z1[:, :, :])
                invZ = zp.tile([NH, N], f32, tag="invZ")
                nc.vector.reciprocal(invZ[:, :], Z12[:, :])
                invZd = dramp.tile([NH, N], f32, tag="invZd")
                nc.sync.dma_start(out=invZd[:, :], in_=invZ[:, :])

                # AV + normalize + relu
                for h in range(NH):
                    rps = ps2.tile([128, N], f32, tag="ps2")
                    for mt2 in range(2):
                        nc.tensor.matmul(
                            rps[:, :],
                            vT_sb[mt2][:, h * 128:(h + 1) * 128],
                            E_sb[mt2][:, h * N:(h + 1) * N],
                            start=(mt2 == 0),
                            stop=(mt2 == 1),
                        )
                    invZb = small.tile([128, N], f32, tag="invZb")
                    nc.sync.dma_start(
                        out=invZb[:, :],
                        in_=invZd[h:h + 1, :].to_broadcast([128, N]))
                    tmp2 = small.tile([128, N], f32, tag="avtmp")
                    nc.vector.tensor_mul(tmp2[:, :], rps[:, :], invZb[:, :])
                    if i == 0:
                        rt = relup.tile([128, G, N], f32, tag=f"relu{h}")
                        relu_t[0][h] = rt
                    else:
                        rt = relu_t[0][h]
                    nc.scalar.activation(
                        rt[:, i, :], tmp2[:, :], AF.Relu, bias=tv_sb[:, h:h + 1])

            # ---------- proj (pair-batched) + BN bias + store ----------
            for mt in range(3):
                mps = ps.tile([128, G * N], f32, tag="ps")
                for kt in range(NH):
                    nc.tensor.matmul(
                        mps[:, :],
                        wp_sb[kt][:, mt * 128:(mt + 1) * 128],
                        relu_t[0][kt][:, :, :],
                        start=(kt == 0),
                        stop=(kt == NH - 1),
                    )
                o_sb = small.tile([128, G * N], bf16, tag="osb")
                nc.scalar.activation(
                    o_sb[:, :], mps[:, :], AF.Identity, bias=tp_sb[:, mt:mt + 1])
                for i in range(G):
                    nc.sync.dma_start(
                        out=out_d[i0 + i, mt * 128:(mt + 1) * 128, :],
                        in_=o_sb[:, i * N:(i + 1) * N],
                    )

    nc.finalize()
    return nc


def _host_prep_weights(inp):
    """Fold BN into weights, build the per-core feed dict (numpy, final dtypes)."""
    import ml_dtypes

    bf16 = ml_dtypes.bfloat16
    s_qkv = inp["qkv_g"] / np.sqrt(inp["qkv_v"] + EPS)
    t_qkv = inp["qkv_b"] - inp["qkv_m"] * s_qkv
    W = inp["qkv_w"][:, :, 0, 0] * s_qkv[:, None]          # [2304, 384]
    Wq = W[:NHKD]
    Wk = W[NHKD:2 * NHKD] * (KD ** -0.5)
    Wv = W[2 * NHKD:]
    tq = t_qkv[:NHKD]
    tv = t_qkv[2 * NHKD:]
    wqkT = np.ascontiguousarray(np.concatenate([Wq, Wk], 0).T)   # [384, 768]
    wvT = np.ascontiguousarray(Wv.T)                             # [384, 1536]

    s_dw = inp["dw_g"] / np.sqrt(inp["dw_v"] + EPS)
    tdw = inp["dw_b"] - inp["dw_m"] * s_dw
    wtap = inp["dw_w"][:, 0].reshape(NHKD, 9) * s_dw[:, None]    # [384, 9]

    s_p = inp["proj_g"] / np.sqrt(inp["proj_v"] + EPS)
    tp = inp["proj_b"] - inp["proj_m"] * s_p
    wpT = np.ascontiguousarray((inp["proj_w"][:, :, 0, 0] * s_p[:, None]).T)

    bias_full = np.take(inp["attn_biases"], inp["bias_idxs"], axis=1)  # [12,n,m]
    bias_m = bias_full.transpose(0, 2, 1)                               # [12,m,n]
    biasT = np.ascontiguousarray(
        bias_m.reshape(NH, 2, MT, N).transpose(1, 2, 0, 3).reshape(2, MT, NH * N))

    def col(v):   # [384] -> [128, 3]
        return np.ascontiguousarray(v.reshape(3, 128).T)

    return {
        "wqkT": wqkT.astype(bf16),
        "wvT": wvT.astype(bf16),
        "wpT": wpT.astype(np.float32),
        "biasT": biasT.astype(np.float32),
        "tq": col(tq).astype(np.float32),
        "tdw": col(tdw).astype(np.float32),
        "wtap": np.ascontiguousarray(
            wtap.reshape(3, 128, 9).transpose(1, 0, 2).reshape(128, 27)
        ).astype(np.float32),
        "tv": np.ascontiguousarray(tv.reshape(NH, 128).T).astype(np.float32),
        "tp": col(tp).astype(np.float32),
    }


_WEIGHT_KEYS = (
    "qkv_w", "qkv_g", "qkv_b", "qkv_m", "qkv_v",
    "dw_w", "dw_g", "dw_b", "dw_m", "dw_v",
    "proj_w", "proj_g", "proj_b", "proj_m", "proj_v",
    "attn_biases", "bias_idxs",
)


def get_nc():
    if "nc" not in _cache:
        _cache["nc"] = _build_nc()
    return _cache["nc"]


def _get_runtime():
    """Build (once) the jitted sharded executable + device plumbing."""
    if "rt" in _cache:
        return _cache["rt"]

    import jax
    import jax.numpy as jnp
    from concourse import bass2jax, mybir
    from jax.sharding import Mesh, PartitionSpec, NamedSharding
    from jax.experimental.shard_map import shard_map

    nc = get_nc()
    bass2jax.install_neuronx_cc_hook()
    assert nc.dbg_addr is None, "kernel must be built with debug=False"

    partition_name = nc.partition_id_tensor.name if nc.partition_id_tensor else None

    in_names = []
    out_names = []
    out_avals = []
    out_np_dtypes = []
    for alloc in nc.m.functions[0].allocations:
        if not isinstance(alloc, mybir.MemoryLocationSet):
            continue
        assert alloc.memorylocations
        name = alloc.memorylocations[0].name
        if alloc.kind == "ExternalInput":
            if name != partition_name:
                in_names.append(name)
        elif alloc.kind == "ExternalOutput":
            assert alloc.tensor_shape is not None and alloc.dtype is not None
            out_names.append(name)
            shape = tuple(alloc.tensor_shape)
            dtype = mybir.dt.np(alloc.dtype)
            out_avals.append(jax.core.ShapedArray(shape, dtype))
            out_np_dtypes.append(dtype)
    n_params = len(in_names)
    n_outs = len(out_avals)
    in_names_full = list(in_names) + list(out_names)
    if partition_name is not None:
        in_names_full.append(partition_name)

    donate = tuple(range(n_params, n_params + n_outs))

    def _body(*args):
        operands = list(args)
        if partition_name is not None:
            operands.append(bass2jax.partition_id_tensor())
        outs = bass2jax._bass_exec_p.bind(
            *operands,
            out_avals=tuple(out_avals),
            in_names=tuple(in_names_full),
            out_names=tuple(out_names),
            lowering_input_output_aliases=(),
            sim_require_finite=True,
            sim_require_nnan=True,
            nc=nc,
        )
        return tuple(outs)

    devices = jax.devices()[:NCORES]
    assert len(devices) == NCORES
    mesh = Mesh(np.asarray(devices), ("core",))
    in_specs = (PartitionSpec("core"),) * (n_params + n_outs)
    out_specs = (PartitionSpec("core"),) * n_outs
    sharded = jax.jit(
        shard_map(
            _body, mesh=mesh, in_specs=in_specs, out_specs=out_specs,
            check_rep=False,
        ),
        donate_argnums=donate,
        keep_unused=True,
    )
    sharding = NamedSharding(mesh, PartitionSpec("core"))

    rt = {
        "sharded": sharded,
        "sharding": sharding,
        "in_names": in_names,
        "out_names": out_names,
        "out_avals": out_avals,
        "w_dev": None,       # name -> device-resident global array
        "w_src": None,       # raw weight inputs the cache was built from
        "next_out_buf": None,  # recycled donated output buffer
    }
    _cache["rt"] = rt
    return rt


def _ensure_weights(rt, inputs):
    """Upload weights once; re-upload only if the weight inputs changed."""
    import jax

    src = {k: np.asarray(inputs[k]) for k in _WEIGHT_KEYS}
    if rt["w_dev"] is not None and all(
        np.array_equal(src[k], rt["w_src"][k]) for k in _WEIGHT_KEYS
    ):
        return
    feed = _host_prep_weights(
        {k: (v.astype(np.float32) if v.dtype != np.int32 else v)
         for k, v in src.items()})
    w_dev = {}
    for name, arr in feed.items():
        glob = np.ascontiguousarray(
            np.broadcast_to(arr[None], (NCORES,) + arr.shape).reshape(
                (NCORES * arr.shape[0],) + arr.shape[1:]))
        w_dev[name] = jax.device_put(glob, rt["sharding"])
    for v in w_dev.values():
        v.block_until_ready()
    rt["w_dev"] = w_dev
    rt["w_src"] = src


def _get_out_buf(rt):
    import jax
    if rt["next_out_buf"] is not None:
        buf = rt["next_out_buf"]
        rt["next_out_buf"] = None
        return buf
    aval = rt["out_avals"][0]
    glob = np.zeros((NCORES * aval.shape[0],) + tuple(aval.shape[1:]),
                    aval.dtype)
    return jax.device_put(glob, rt["sharding"])


def kernel(**inputs) -> np.ndarray:
    import ml_dtypes

    rt = _get_runtime()
    _ensure_weights(rt, inputs)

    x = np.asarray(inputs["x"], dtype=np.float32).reshape(B, C, N)
    xb = x.astype(ml_dtypes.bfloat16)   # global [128, C, N] == concat of shards

    args = []
    for name in rt["in_names"]:
        if name == "x":
            args.append(xb)
        else:
            args.append(rt["w_dev"][name])
    args.append(_get_out_buf(rt))

    (out_g,) = rt["sharded"](*args)
    out = np.asarray(out_g)
    rt["next_out_buf"] = out_g
    return out.astype(np.float32).reshape(B, C, RES, RES)


# revision 3
# speedup vs baseline: 6.6031x; 1.6214x over previous
"""Trainium2 Bass kernel for nn_Attention_75453985457143 (EfficientViT-style
attention block: 1x1 conv QKV + BN, depthwise 3x3 on Q + BN, MHSA with relative
position bias, ReLU, 1x1 proj + BN).

Data-parallel over batch: 128 images -> 16 per NeuronCore across 8 cores.
All BN affine transforms are folded into weights/bias vectors on the host.

The wall-clock cost of a call is dominated by host<->device transfer over the
axon relay (~55-65 MB/s, serialized), so the runtime path is organized around
minimizing transferred bytes per call:
  - x is uploaded in bf16 (19.25 MB instead of 38.5 MB f32)
  - the output comes back in bf16 and is cast to f32 on the host
  - all weights are uploaded to the devices once and kept resident
  - the donated output buffer is recycled from the previous call's output
    instead of uploading fresh zero buffers every call (the kernel writes
    every element of out, so the initial contents are irrelevant)
"""

import time
import numpy as np

# ---- problem constants (hardcoded; kernel.py must be self-contained) ----
B = 128
C = 384
KD = 32
NH = 12
NHKD = 384          # q/k channels
DH = 1536           # v channels
RES = 14
N = RES * RES       # 196 tokens
EPS = 1e-5
NCORES = 8
BPC = B // NCORES   # 16 images per core
G = 2               # images per group (pair)
NG = BPC // G       # 8 groups
MT = 98             # attention m-tile (2 tiles of 98 = 196)

_cache = {}


def _build_nc(ng=NG):
    import concourse.bacc as bacc
    import concourse.tile as tile
    from concourse import mybir
    from concourse.alu_op_type import AluOpType
    from contextlib import ExitStack

    f32 = mybir.dt.float32
    bf16 = mybir.dt.bfloat16
    AF = mybir.ActivationFunctionType

    nc = bacc.Bacc("TRN2", target_bir_lowering=False, debug=False, num_devices=NCORES)

    # ---- DRAM I/O ----
    x_d = nc.dram_tensor("x", [BPC, C, N], bf16, kind="ExternalInput")
    wqk_d = nc.dram_tensor("wqkT", [C, 2 * NHKD], bf16, kind="ExternalInput")
    wv_d = nc.dram_tensor("wvT", [C, DH], bf16, kind="ExternalInput")
    wp_d = nc.dram_tensor("wpT", [DH, C], f32, kind="ExternalInput")
    biasT_d = nc.dram_tensor("biasT", [2, MT, NH * N], f32, kind="ExternalInput")
    tq_d = nc.dram_tensor("tq", [128, 3], f32, kind="ExternalInput")
    tdw_d = nc.dram_tensor("tdw", [128, 3], f32, kind="ExternalInput")
    wtap_d = nc.dram_tensor("wtap", [128, 27], f32, kind="ExternalInput")
    tv_d = nc.dram_tensor("tv", [128, NH], f32, kind="ExternalInput")
    tp_d = nc.dram_tensor("tp", [128, 3], f32, kind="ExternalInput")
    out_d = nc.dram_tensor("out", [BPC, C, N], bf16, kind="ExternalOutput")

    with tile.TileContext(nc) as tc, ExitStack() as ctx:
        singles = ctx.enter_context(tc.tile_pool(name="singles", bufs=1))
        grp2 = ctx.enter_context(tc.tile_pool(name="grp2", bufs=2))
        grp1 = ctx.enter_context(tc.tile_pool(name="grp1", bufs=1))
        imgp = ctx.enter_context(tc.tile_pool(name="imgp", bufs=2))
        accp = ctx.enter_context(tc.tile_pool(name="accp", bufs=1))
        zp = ctx.enter_context(tc.tile_pool(name="zp", bufs=1))
        small = ctx.enter_context(tc.tile_pool(name="small", bufs=3))
        regp = ctx.enter_context(tc.tile_pool(name="regp", bufs=1))
        relup = ctx.enter_context(tc.tile_pool(name="relup", bufs=1))
        ps = ctx.enter_context(tc.tile_pool(name="ps", bufs=2, space="PSUM"))
        ps2 = ctx.enter_context(tc.tile_pool(name="ps2", bufs=6, space="PSUM"))
        dramp = ctx.enter_context(tc.tile_pool(name="dramp", bufs=2, space="DRAM"))

        # ---- persistent constants ----
        wqk_sb = []
        wv_sb = []
        for kt in range(3):
            t = singles.tile([128, 2 * NHKD], bf16, tag=f"wqk{kt}")
            nc.sync.dma_start(out=t[:, :], in_=wqk_d[kt * 128:(kt + 1) * 128, :])
            wqk_sb.append(t)
            t = singles.tile([128, DH], bf16, tag=f"wv{kt}")
            nc.sync.dma_start(out=t[:, :], in_=wv_d[kt * 128:(kt + 1) * 128, :])
            wv_sb.append(t)
        wp_sb = []
        for kt in range(NH):
            t = singles.tile([128, C], f32, tag=f"wp{kt}")
            nc.sync.dma_start(out=t[:, :], in_=wp_d[kt * 128:(kt + 1) * 128, :])
            wp_sb.append(t)
        biasT_sb = []
        for mt2 in range(2):
            t = singles.tile([MT, NH * N], f32, tag=f"biasT{mt2}")
            nc.sync.dma_start(out=t[:, :], in_=biasT_d[mt2])
            biasT_sb.append(t)
        tq_sb = singles.tile([128, 3], f32, tag="tq")
        nc.sync.dma_start(out=tq_sb[:, :], in_=tq_d[:, :])
        tdw_sb = singles.tile([128, 3], f32, tag="tdw")
        nc.sync.dma_start(out=tdw_sb[:, :], in_=tdw_d[:, :])
        wtap_sb = singles.tile([128, 27], f32, tag="wtap")
        nc.sync.dma_start(out=wtap_sb[:, :], in_=wtap_d[:, :])
        tv_sb = singles.tile([128, NH], f32, tag="tv")
        nc.sync.dma_start(out=tv_sb[:, :], in_=tv_d[:, :])
        tp_sb = singles.tile([128, 3], f32, tag="tp")
        nc.sync.dma_start(out=tp_sb[:, :], in_=tp_d[:, :])
        ones98 = singles.tile([MT, 1], bf16, tag="ones98")
        nc.vector.memset(ones98[:, :], 1.0)

        for g in range(ng):
            i0 = g * G
            # ---------- phase A: load x, qkv matmuls ----------
            x_sb = []
            for kt in range(3):
                t = grp2.tile([128, G, N], bf16, tag=f"x{kt}")
                nc.sync.dma_start(
                    out=t[:, :, :],
                    in_=x_d[i0:i0 + G, kt * 128:(kt + 1) * 128, :].rearrange(
                        "g c n -> c g n"),
                )
                x_sb.append(t)
            k_sb = []
            qpad = []
            for pt in range(3):
                t = grp2.tile([128, G, N], bf16, tag=f"k{pt}")
                k_sb.append(t)
                t = grp1.tile([128, G, 16, 16], f32, tag=f"qpad{pt}")
                nc.vector.memset(t[:, :, :, :], 0.0)
                qpad.append(t)

            for mt in range(6):
                qk_ps = ps.tile([128, G * N], f32, tag="ps")
                for kt in range(3):
                    nc.tensor.matmul(
                        qk_ps[:, :],
                        wqk_sb[kt][:, mt * 128:(mt + 1) * 128],
                        x_sb[kt][:, :, :],
                        start=(kt == 0),
                        stop=(kt == 2),
                    )
                if mt < 3:
                    # q: add BN bias, write into padded interior
                    for i in range(G):
                        nc.scalar.activation(
                            qpad[mt][:, i, 1:15, 1:15],
                            qk_ps[:, i * N:(i + 1) * N].rearrange(
                                "p (a b) -> p a b", a=RES),
                            AF.Identity,
                            bias=tq_sb[:, mt:mt + 1],
                        )
                else:
                    nc.any.tensor_copy(
                        k_sb[mt - 3][:, :, :],
                        qk_ps[:, :].rearrange("p (g n) -> p g n", g=G),
                    )

            # ---------- phase B: depthwise 3x3 conv on q ----------
            qconv = []
            for pt in range(3):
                qc = grp1.tile([128, G, RES, RES], bf16, tag=f"qconv{pt}")
                for i in range(G):
                    acc_prev = None
                    for j in range(9):
                        jr, jc = j // 3, j % 3
                        win = qpad[pt][:, i, jr:jr + RES, jc:jc + RES]
                        w_ap = wtap_sb[:, pt * 9 + j:pt * 9 + j + 1]
                        if j == 8:
                            dst = qc[:, i]
                        else:
                            acc_t = accp.tile([128, RES, RES], f32,
                                              tag=f"acc{pt}_{j % 2}")
                            dst = acc_t[:, :, :]
                        if j == 0:
                            nc.vector.tensor_scalar(
                                dst, win, w_ap,
                                tdw_sb[:, pt:pt + 1],
                                AluOpType.mult, AluOpType.add)
                        else:
                            nc.vector.scalar_tensor_tensor(
                                dst, win, w_ap, acc_prev,
                                AluOpType.mult, AluOpType.add)
                        acc_prev = dst
                qconv.append(qc)

            # ---------- regroup k/qconv to base-partition-0 head layout ----------
            k2 = regp.tile([32, NH, G, N], bf16, tag="k2")
            q2 = regp.tile([32, NH, G, N], bf16, tag="q2")
            for pt in range(3):
                for r in range(4):
                    h = 4 * pt + r
                    nc.sync.dma_start(
                        out=k2[:, h, :, :],
                        in_=k_sb[pt][32 * r:32 * r + 32, :, :])
                    nc.sync.dma_start(
                        out=q2[:, h, :, :],
                        in_=qconv[pt][32 * r:32 * r + 32, :, :, :].rearrange(
                            "d g a b -> d g (a b)"))

            # ---------- phase C: per-image attention ----------
            relu_t = [[None] * NH for _ in range(G)]
            for i in range(G):
                # v^T: [196, 1536] via x-stationary matmuls
                vT_sb = []
                for mt2 in range(2):
                    vt = imgp.tile([MT, DH], bf16, tag=f"vT{mt2}")
                    for ch in range(3):
                        vps = ps.tile([MT, 512], f32, tag="ps")
                        for kt in range(3):
                            nc.tensor.matmul(
                                vps[:, :],
                                x_sb[kt][:, i, mt2 * MT:(mt2 + 1) * MT],
                                wv_sb[kt][:, ch * 512:(ch + 1) * 512],
                                start=(kt == 0),
                                stop=(kt == 2),
                            )
                        nc.any.tensor_copy(vt[:, ch * 512:(ch + 1) * 512], vps[:, :])
                    vT_sb.append(vt)

                # QK + bias + exp (E^T layout [m, n], head pairs packed in free)
                E_sb = []
                for mt2 in range(2):
                    et = imgp.tile([MT, NH * N], bf16, tag=f"E{mt2}")
                    E_sb.append(et)
                for mt2 in range(2):
                    for hp in range(6):
                        sps = ps2.tile([MT, 2 * N], f32, tag="ps2")
                        for hh in range(2):
                            h = 2 * hp + hh
                            nc.tensor.matmul(
                                sps[:, hh * N:(hh + 1) * N],
                                k2[:, h, i, mt2 * MT:(mt2 + 1) * MT],
                                q2[:, h, i, :],
                                start=True,
                                stop=True,
                            )
                        tmp = small.tile([MT, 2 * N], f32, tag="stmp")
                        nc.vector.tensor_add(
                            tmp[:, :], sps[:, :],
                            biasT_sb[mt2][:, hp * 2 * N:(hp + 1) * 2 * N])
                        nc.scalar.activation(
                            E_sb[mt2][:, hp * 2 * N:(hp + 1) * 2 * N],
                            tmp[:, :], AF.Exp)

                # Z = colsums of E (per head) via ones-stationary matmuls
                Z1 = zp.tile([1, NH, N], f32, tag="Z1")
                for hp in range(6):
                    zps = ps2.tile([1, 2 * N], f32, tag="ps2")
                    for hh in range(2):
                        h = 2 * hp + hh
                        for mt2 in range(2):
                            nc.tensor.matmul(
                                zps[:, hh * N:(hh + 1) * N],
                                ones98[:, :],
                                E_sb[mt2][:, h * N:(h + 1) * N],
                                start=(mt2 == 0),
                                stop=(mt2 == 1),
                            )
                    nc.any.tensor_copy(
                        Z1[:, 2 * hp:2 * hp + 2, :],
                        zps[:, :].rearrange("p (a n) -> p a n", a=2))
                # shuffle [1, 12*196] -> [12, 196] so reciprocal gets 12 lanes
                Z12 = zp.tile([NH, N], f32, tag="Z12")
                nc.sync.dma_start(out=Z12[:, :], in_=Z1[:, :, :])
                invZ = zp.tile([NH, N], f32, tag="invZ")
                nc.vector.reciprocal(invZ[:, :], Z12[:, :])
                invZd = dramp.tile([NH, N], f32, tag="invZd")
                nc.sync.dma_start(out=invZd[:, :], in_=invZ[:, :])

                # AV + normalize + relu
                for h in range(NH):
                    rps = ps2.tile([128, N], f32, tag="ps2")
                    for mt2 in range(2):
                        nc.tensor.matmul(
                            rps[:, :],
                            vT_sb[mt2][:, h * 128:(h + 1) * 128],
                            E_sb[mt2][:, h * N:(h + 1) * N],
                            start=(mt2 == 0),
                            stop=(mt2 == 1),
                        )
                    invZb = small.tile([128, N], f32, tag="invZb")
                    nc.sync.dma_start(
                        out=invZb[:, :],
                        in_=invZd[h:h + 1, :].to_broadcast([128, N]))
                    tmp2 = small.tile([128, N], f32, tag="avtmp")
                    nc.vector.tensor_mul(tmp2[:, :], rps[:, :], invZb[:, :])
                    if i == 0:
                        rt = relup.tile([128, G, N], f32, tag=f"relu{h}")
                        relu_t[0][h] = rt
                    else:
                        rt = relu_t[0][h]
                    nc.scalar.activation(
                        rt[:, i, :], tmp2[:, :], AF.Relu, bias=tv_sb[:, h:h + 1])

            # ---------- proj (pair-batched) + BN bias + store ----------
            for mt in range(3):
                mps = ps.tile([128, G * N], f32, tag="ps")
                for kt in range(NH):
                    nc.tensor.matmul(
                        mps[:, :],
                        wp_sb[kt][:, mt * 128:(mt + 1) * 128],
                        relu_t[0][kt][:, :, :],
                        start=(kt == 0),
                        stop=(kt == NH - 1),
                    )
                o_sb = small.tile([128, G * N], bf16, tag="osb")
                nc.scalar.activation(
                    o_sb[:, :], mps[:, :], AF.Identity, bias=tp_sb[:, mt:mt + 1])
                for i in range(G):
                    nc.sync.dma_start(
                        out=out_d[i0 + i, mt * 128:(mt + 1) * 128, :],
                        in_=o_sb[:, i * N:(i + 1) * N],
                    )

    nc.finalize()
    return nc


def _host_prep_weights(inp):
    """Fold BN into weights, build the per-core feed dict (numpy, final dtypes)."""
    import ml_dtypes

    bf16 = ml_dtypes.bfloat16
    s_qkv = inp["qkv_g"] / np.sqrt(inp["qkv_v"] + EPS)
    t_qkv = inp["qkv_b"] - inp["qkv_m"] * s_qkv
    W = inp["qkv_w"][:, :, 0, 0] * s_qkv[:, None]          # [2304, 384]
    Wq = W[:NHKD]
    Wk = W[NHKD:2 * NHKD] * (KD ** -0.5)
    Wv = W[2 * NHKD:]
    tq = t_qkv[:NHKD]
    tv = t_qkv[2 * NHKD:]
    wqkT = np.ascontiguousarray(np.concatenate([Wq, Wk], 0).T)   # [384, 768]
    wvT = np.ascontiguousarray(Wv.T)                             # [384, 1536]

    s_dw = inp["dw_g"] / np.sqrt(inp["dw_v"] + EPS)
    tdw = inp["dw_b"] - inp["dw_m"] * s_dw
    wtap = inp["dw_w"][:, 0].reshape(NHKD, 9) * s_dw[:, None]    # [384, 9]

    s_p = inp["proj_g"] / np.sqrt(inp["proj_v"] + EPS)
    tp = inp["proj_b"] - inp["proj_m"] * s_p
    wpT = np.ascontiguousarray((inp["proj_w"][:, :, 0, 0] * s_p[:, None]).T)

    bias_full = np.take(inp["attn_biases"], inp["bias_idxs"], axis=1)  # [12,n,m]
    bias_m = bias_full.transpose(0, 2, 1)                               # [12,m,n]
    biasT = np.ascontiguousarray(
        bias_m.reshape(NH, 2, MT, N).transpose(1, 2, 0, 3).reshape(2, MT, NH * N))

    def col(v):   # [384] -> [128, 3]
        return np.ascontiguousarray(v.reshape(3, 128).T)

    return {
        "wqkT": wqkT.astype(bf16),
        "wvT": wvT.astype(bf16),
        "wpT": wpT.astype(np.float32),
        "biasT": biasT.astype(np.float32),
        "tq": col(tq).astype(np.float32),
        "tdw": col(tdw).astype(np.float32),
        "wtap": np.ascontiguousarray(
            wtap.reshape(3, 128, 9).transpose(1, 0, 2).reshape(128, 27)
        ).astype(np.float32),
        "tv": np.ascontiguousarray(tv.reshape(NH, 128).T).astype(np.float32),
        "tp": col(tp).astype(np.float32),
    }


_WEIGHT_KEYS = (
    "qkv_w", "qkv_g", "qkv_b", "qkv_m", "qkv_v",
    "dw_w", "dw_g", "dw_b", "dw_m", "dw_v",
    "proj_w", "proj_g", "proj_b", "proj_m", "proj_v",
    "attn_biases", "bias_idxs",
)


def get_nc():
    if "nc" not in _cache:
        _cache["nc"] = _build_nc()
    return _cache["nc"]


def _get_runtime():
    """Build (once) the jitted sharded executable + device plumbing."""
    if "rt" in _cache:
        return _cache["rt"]

    import jax
    import jax.numpy as jnp
    from concourse import bass2jax, mybir
    from jax.sharding import Mesh, PartitionSpec, NamedSharding
    from jax.experimental.shard_map import shard_map

    nc = get_nc()
    bass2jax.install_neuronx_cc_hook()
    assert nc.dbg_addr is None, "kernel must be built with debug=False"

    partition_name = nc.partition_id_tensor.name if nc.partition_id_tensor else None

    in_names = []
    out_names = []
    out_avals = []
    out_np_dtypes = []
    for alloc in nc.m.functions[0].allocations:
        if not isinstance(alloc, mybir.MemoryLocationSet):
            continue
        assert alloc.memorylocations
        name = alloc.memorylocations[0].name
        if alloc.kind == "ExternalInput":
            if name != partition_name:
                in_names.append(name)
        elif alloc.kind == "ExternalOutput":
            assert alloc.tensor_shape is not None and alloc.dtype is not None
            out_names.append(name)
            shape = tuple(alloc.tensor_shape)
            dtype = mybir.dt.np(alloc.dtype)
            out_avals.append(jax.core.ShapedArray(shape, dtype))
            out_np_dtypes.append(dtype)
    n_params = len(in_names)
    n_outs = len(out_avals)
    in_names_full = list(in_names) + list(out_names)
    if partition_name is not None:
        in_names_full.append(partition_name)

    donate = tuple(range(n_params, n_params + n_outs))

    def _body(*args):
        operands = list(args)
        if partition_name is not None:
            operands.append(bass2jax.partition_id_tensor())
        outs = bass2jax._bass_exec_p.bind(
            *operands,
            out_avals=tuple(out_avals),
            in_names=tuple(in_names_full),
            out_names=tuple(out_names),
            lowering_input_output_aliases=(),
            sim_require_finite=True,
            sim_require_nnan=True,
            nc=nc,
        )
        return tuple(outs)

    devices = jax.devices()[:NCORES]
    assert len(devices) == NCORES
    mesh = Mesh(np.asarray(devices), ("core",))
    in_specs = (PartitionSpec("core"),) * (n_params + n_outs)
    out_specs = (PartitionSpec("core"),) * n_outs
    sharded = jax.jit(
        shard_map(
            _body, mesh=mesh, in_specs=in_specs, out_specs=out_specs,
            check_rep=False,
        ),
        donate_argnums=donate,
        keep_unused=True,
    )
    sharding = NamedSharding(mesh, PartitionSpec("core"))

    rt = {
        "sharded": sharded,
        "sharding": sharding,
        "in_names": in_names,
        "out_names": out_names,
        "out_avals": out_avals,
        "w_dev": None,       # name -> device-resident global array
        "w_src": None,       # raw weight inputs the cache was built from
        "next_out_buf": None,  # recycled donated output buffer
    }
    _cache["rt"] = rt
    return rt


def _ensure_weights(rt, inputs):
    """Upload weights once; re-upload only if the weight inputs changed."""
    import jax

    src = {k: np.asarray(inputs[k]) for k in _WEIGHT_KEYS}
    if rt["w_dev"] is not None and all(
        np.array_equal(src[k], rt["w_src"][k]) for k in _WEIGHT_KEYS
    ):
        return
    feed = _host_prep_weights(
        {k: (v.astype(np.float32) if v.dtype != np.int32 else v)
         for k, v in src.items()})
    w_dev = {}
    for name, arr in feed.items():
        glob = np.ascontiguousarray(
            np.broadcast_to(arr[None], (NCORES,) + arr.shape).reshape(
                (NCORES * arr.shape[0],) + arr.shape[1:]))
        w_dev[name] = jax.device_put(glob, rt["sharding"])
    for v in w_dev.values():
        v.block_until_ready()
    rt["w_dev"] = w_dev
    rt["w_src"] = src


def _get_out_buf(rt):
    import jax
    if rt["next_out_buf"] is not None:
        buf = rt["next_out_buf"]
        rt["next_out_buf"] = None
        return buf
    aval = rt["out_avals"][0]
    glob = np.zeros((NCORES * aval.shape[0],) + tuple(aval.shape[1:]),
                    aval.dtype)
    return jax.device_put(glob, rt["sharding"])


def kernel(**inputs) -> np.ndarray:
    import os
    import ml_dtypes

    dbg = os.environ.get("KERNEL_TIMING") == "1"
    tmarks = [("start", time.perf_counter())]

    rt = _get_runtime()
    tmarks.append(("runtime", time.perf_counter()))
    _ensure_weights(rt, inputs)
    tmarks.append(("weights", time.perf_counter()))

    x = np.asarray(inputs["x"], dtype=np.float32).reshape(B, C, N)
    xb = x.astype(ml_dtypes.bfloat16)   # global [128, C, N] == concat of shards
    tmarks.append(("cast_x", time.perf_counter()))

    args = []
    for name in rt["in_names"]:
        if name == "x":
            args.append(xb)
        else:
            args.append(rt["w_dev"][name])
    args.append(_get_out_buf(rt))
    tmarks.append(("argprep", time.perf_counter()))

    (out_g,) = rt["sharded"](*args)
    tmarks.append(("dispatch", time.perf_counter()))
    out = np.asarray(out_g)
    tmarks.append(("fetch", time.perf_counter()))
    rt["next_out_buf"] = out_g
    res = out.astype(np.float32).reshape(B, C, RES, RES)
    tmarks.append(("cast_out", time.perf_counter()))
    if dbg:
        import sys
        parts = " ".join(
            f"{tmarks[i][0]}={1e3 * (tmarks[i][1] - tmarks[i - 1][1]):.0f}ms"
            for i in range(1, len(tmarks)))
        print(f"[kernel timing] {parts}", file=sys.stderr)
    return res
